# revision 15
# baseline (speedup 1.0000x reference)
"""TRN2 Bass kernel for nn_BrainModule (sparse_attention).

Computation (per sample b):
  emb[c,d]   = fourier embedding of positions[b,c]          (d = 242)
  scores[o,c]= heads[subj[b]][o,:] . emb[c,:] + offset[c]   (offset = -1e9 on
                                                             invalid channels)
  w[o,c]     = softmax_c(scores)
  out[o,t]   = sum_c w[o,c] * meg[b,c,t]

Data-parallel over batch B=32 across 8 cores (4 samples each).

Fast path (taken when the invalid channels are exactly the common suffix
257..272, which the module always produces):
  - Embeddings via a K=3 PE outer-product matmul (stationary [fi, fj, shift]
    columns x moving [pa, pb, 1] rows), int-cast range reduction, ACT Sin.
    No 121-way broadcast DMAs.
  - Scores in fp16 with emb/heads K-padded to 128 so FWL (fast weight load)
    engages; softmax sums via a ones-stationary matmul -> [1, 270], then a
    K=1 replicate matmul + DVE reciprocal; weights pre-scaled by 1/sum on
    DVE so the big-matmul PSUM->SBUF copies are plain copies.
  - The big einsum in fp16 (more mantissa than bf16 at the same byte cost),
    output chout rows 0..255 as two 128-row chunks per sample; the awkward
    14-row tail chunk (256..270) of all 4 samples is packed into 4
    concurrent PE column-tiles (tile_position=(0,32b)), and the K=1 channel
    rides diagonal (32b,32b) tiles -- ~3x less PE time for that chunk.
  - DMA split across three queues: sync HWDGE (meg chunk 0), scalar HWDGE
    (heads + meg chunk 1), gpsimd SWDGE (most stores); late stores spill to
    the by-then-idle HWDGE queues.
"""
import numpy as np

B, C, T = 32, 273, 4096
CHOUT = 270
N_FREQS = 11
NF2 = N_FREQS * N_FREQS          # 121
D_A = NF2 + 1                    # cos half + offset/ones row (fallback path)
MARGIN = 0.2
WIDTH = 1.0 + 2.0 * MARGIN
INVALID = -0.1
NEG_INF = -1e9
N_CORES = 8
BS = B // N_CORES                # samples per core
CB = BS * C                      # batched embedding width (1092)
PW = CB + 2 * NF2                # pos3 packed width (1334)
C_USED_FAST = 257                # fast-path channel prefix
TWO_PI = float(2.0 * np.pi)
# largest f32 <= 2*pi, so |frac| = 0.5 never maps beyond pi
SCALE_2PI = float(np.nextafter(np.float32(2.0 * np.pi), np.float32(0.0)))

M_CHUNKS = [(0, 128), (128, 128), (256, CHOUT - 256)]  # partition chunks of O
TH = 2048                                              # meg/out tile t width
NT_Q = TH // 512                                       # 512-wide psum tiles

_NC_CACHE = {}


# --------------------------------------------------------------------------
# fast-path builder
# --------------------------------------------------------------------------

def _build_fast():
    import concourse.bacc as bacc
    import concourse.mybir as mybir
    import concourse.tile as tile

    F32 = mybir.dt.float32
    F32R = mybir.dt.float32r
    F16 = mybir.dt.float16
    I32 = mybir.dt.int32
    Sin = mybir.ActivationFunctionType.Sin
    Exp = mybir.ActivationFunctionType.Exp
    Copy = mybir.ActivationFunctionType.Copy

    NTH = T // TH                # 2

    nc = bacc.Bacc("TRN2", target_bir_lowering=False, debug=False,
                   num_devices=N_CORES)

    meg_d = nc.dram_tensor("meg", [BS, C_USED_FAST, T], F16,
                           kind="ExternalInput")
    pos3_d = nc.dram_tensor("pos3", [3, PW], F32R, kind="ExternalInput")
    onesr_d = nc.dram_tensor("onesr", [1, 128], F32R, kind="ExternalInput")
    hh_d = nc.dram_tensor("hh", [BS, 128, 2 * CHOUT], F16,
                          kind="ExternalInput")
    out_d = nc.dram_tensor("out", [BS, CHOUT, T], F16, kind="ExternalOutput")

    with tile.TileContext(nc) as tc:
        with (
            tc.tile_pool(name="const", bufs=1) as const,
            tc.tile_pool(name="embsb", bufs=1) as embsb,
            tc.tile_pool(name="embw", bufs=2) as embw,
            tc.tile_pool(name="wsb", bufs=1) as wsb,
            tc.tile_pool(name="wraw", bufs=2) as wraw,
            tc.tile_pool(name="persist", bufs=1) as persist,
            tc.tile_pool(name="megp", bufs=1) as megp,
            tc.tile_pool(name="outp", bufs=3) as outp,
            tc.tile_pool(name="pp", bufs=1, space="PSUM") as pp,
        ):
            # ---- input DMAs ------------------------------------------------
            # consolidated: one DMA per meg sample (both 128-chunks), one DMA
            # for all k1-channel layouts, one combined heads DMA per sample.
            # sync queue: pos3 + meg 0/1 + k1 layouts; scalar queue: heads +
            # meg 2/3 (so neither engine burns long on DMA issue).
            pos3 = const.tile([3, PW], F32R, tag="pos3")
            nc.sync.dma_start(out=pos3, in_=pos3_d[:, :])

            ones_col = const.tile([128, 1], F16, tag="ones_col")
            nc.gpsimd.memset(ones_col, 1.0)
            ones_row = const.tile([1, 128], F32R, tag="ones_row")
            nc.sync.dma_start(out=ones_row, in_=onesr_d[:, :])

            hhs = []
            for b in range(BS):
                hh = wsb.tile([128, 2 * CHOUT], F16, tag=f"hh_{b}")
                nc.scalar.dma_start(out=hh, in_=hh_d[b, :, :])
                hhs.append(hh)

            megs = []
            for b in range(BS):
                q = nc.sync if b < 2 else nc.scalar
                mg = megp.tile([128, 2 * T], F16, tag=f"mg_{b}")
                src = meg_d[b, 0:256, :].rearrange("(k p) f -> p k f", k=2)
                dst = mg.rearrange("p (k f) -> p k f", k=2)
                q.dma_start(out=dst, in_=src)
                megs.append([mg[:, 0:T], mg[:, T:2 * T]])

            # k1 channel, tq-quadrant layout for the per-sample K1 wave:
            # partition 32q holds [th, b, 512] for t-chunk q (th-major so
            # each DMA stays within the 3-dim AP limit)
            mgr = megp.tile([97, NTH * BS * 512], F16, tag="mgr")
            for th in range(NTH):
                src = meg_d[:, 256, th * TH:(th + 1) * TH].rearrange(
                    "b (q f) -> q b f", q=NT_Q)
                dst = mgr[0:97:32, th * BS * 512:(th + 1) * BS * 512
                          ].rearrange("q (b f) -> q b f", b=BS)
                nc.sync.dma_start(out=dst, in_=src)
            # k1 channel, sample-quadrant layout for the m3 diagonal
            mgr2 = megp.tile([97, T], F16, tag="mgr2")
            nc.sync.dma_start(out=mgr2[0:97:32, :], in_=meg_d[:, 256, :])

            # ---- embeddings -------------------------------------------------
            # embA = cos half, embB = sin half, channels of all samples along
            # the free dim; rows 121..127 zero so K=128 score matmuls get FWL
            embA = embsb.tile([128, CB], F16, tag="embA")
            embB = embsb.tile([128, CB], F16, tag="embB")
            # memset partition start must be 32-aligned; rows 96..120 are
            # overwritten by the ACT sin writes afterwards (WAW-ordered)
            nc.vector.memset(embA[96:128, :], 0.0)
            nc.vector.memset(embB[96:128, :], 0.0)

            fifi = [pos3[:, CB:CB + NF2],            # cos: [fi, fj, 0.25]
                    pos3[:, CB + NF2:CB + 2 * NF2]]  # sin: [fi, fj, 0]
            embT = [embA, embB]

            def emb_stage(half, w0, wn):
                ps_e = pp.tile([NF2, wn], F32, tag="ws", bufs=3, name="ps_e")
                nc.tensor.matmul(ps_e, fifi[half], pos3[:, w0:w0 + wn],
                                 start=True, stop=True)
                # range reduction to [-0.5, 0.5]: HW f32->i32 cast rounds to
                # nearest, so frac = x - round(x)
                ki = embw.tile([NF2, wn], I32, tag="ki")
                kf = embw.tile([NF2, wn], F32, tag="kf")
                fr = embw.tile([NF2, wn], F32, tag="fr")
                nc.vector.tensor_copy(ki, ps_e)
                nc.vector.tensor_copy(kf, ki)
                nc.vector.tensor_sub(out=fr, in0=ps_e, in1=kf)
                nc.scalar.activation(out=embT[half][0:NF2, w0:w0 + wn],
                                     in_=fr, func=Sin, scale=SCALE_2PI)

            # ---- weight stage: scores -> softmax weights, pre-normalized ---
            wtns, wreps = [], []

            def weight_stage(b):
                co = b * C
                hta = hhs[b][:, 0:CHOUT]
                htb = hhs[b][:, CHOUT:2 * CHOUT]
                ps_s0 = pp.tile([128, CHOUT], F32, tag="ws", bufs=3)
                nc.tensor.matmul(ps_s0, embA[:, co:co + 128], hta,
                                 start=True, stop=False)
                nc.tensor.matmul(ps_s0, embB[:, co:co + 128], htb,
                                 start=False, stop=True)
                ps_s1 = pp.tile([128, CHOUT], F32, tag="ws", bufs=3)
                nc.tensor.matmul(ps_s1, embA[:, co + 128:co + 256], hta,
                                 start=True, stop=False)
                nc.tensor.matmul(ps_s1, embB[:, co + 128:co + 256], htb,
                                 start=False, stop=True)
                ps_s2 = pp.tile([1, CHOUT], F32, tag="ws", bufs=3)
                nc.tensor.matmul(ps_s2, embA[:, co + 256:co + 257], hta,
                                 start=True, stop=False)
                nc.tensor.matmul(ps_s2, embB[:, co + 256:co + 257], htb,
                                 start=False, stop=True)

                wt0 = wraw.tile([128, CHOUT], F16, tag="wt0")
                wt1 = wraw.tile([128, CHOUT], F16, tag="wt1")
                ek1 = wraw.tile([1, CHOUT], F32, tag="ek1")
                nc.scalar.activation(out=wt0, in_=ps_s0, func=Exp)
                nc.scalar.activation(out=wt1, in_=ps_s1, func=Exp)
                nc.scalar.activation(out=ek1, in_=ps_s2, func=Exp)

                ps_sum = pp.tile([1, CHOUT], F32, tag="ws", bufs=3)
                nc.tensor.matmul(ps_sum, ones_col, wt0,
                                 start=True, stop=False)
                nc.tensor.matmul(ps_sum, ones_col, wt1,
                                 start=False, stop=True)
                sums = wraw.tile([1, CHOUT], F32R, tag="sums")
                nc.vector.tensor_add(out=sums, in0=ps_sum, in1=ek1)
                ps_rep = pp.tile([128, CHOUT], F32, tag="ws", bufs=3)
                nc.tensor.matmul(ps_rep, ones_row, sums,
                                 start=True, stop=True)
                invrep = wraw.tile([128, CHOUT], F16, tag="invrep")
                with nc.allow_low_precision(
                        reason="1/sum in f16: 5e-4 rel vs 2e-2 budget"):
                    nc.vector.reciprocal(out=invrep, in_=ps_rep)

                wtn0 = persist.tile([128, CHOUT], F16, tag=f"wtn0_{b}")
                wtn1 = persist.tile([128, CHOUT], F16, tag=f"wtn1_{b}")
                wrep = persist.tile([97, CHOUT], F16, tag=f"wrep_{b}")
                nc.vector.tensor_mul(out=wtn0, in0=wt0, in1=invrep)
                nc.vector.tensor_mul(out=wtn1, in0=wt1, in1=invrep)
                wk1 = wraw.tile([1, CHOUT], F32R, tag="wk1")
                nc.vector.tensor_mul(out=wk1, in0=ek1, in1=invrep[0:1, :])
                # replicate the k1-channel weights to partitions 0..96 via a
                # K=1 matmul (keeps ACT on a pure Sin->Exp->Copy table order)
                ps_rp2 = pp.tile([128, CHOUT], F32, tag="ws", bufs=3)
                nc.tensor.matmul(ps_rp2, ones_row, wk1,
                                 start=True, stop=True)
                nc.vector.tensor_copy(wrep, ps_rp2[0:97, :])
                wtns.append([wtn0, wtn1])
                wreps.append(wrep)

            # all sins, then all exps, then phase-2 copies: 3 ACT table loads
            # total. fp32r matmul ISA restriction: moving width must be even.
            emb_stage(0, 0, C + 1)
            emb_stage(1, 0, C + 1)
            emb_stage(0, C + 1, 512)
            emb_stage(0, C + 513, CB - C - 513)
            emb_stage(1, C + 1, 512)
            emb_stage(1, C + 513, CB - C - 513)
            for b in range(BS):
                weight_stage(b)

            # ---- phase 2a: per-sample big matmuls (chout rows 0..255) ------
            def store_queue(b, mi):
                # gpsimd early; late stores spill to the by-then-idle HWDGE
                # queues (their load FIFOs have drained)
                if b <= 1:
                    return nc.gpsimd
                if b == 2:
                    return nc.gpsimd if mi == 0 else nc.sync
                return nc.scalar if mi == 0 else nc.sync

            for b in range(BS):
                for th in range(NTH):
                    t0 = th * TH
                    for mi in range(2):
                        m0 = mi * 128
                        ot = outp.tile([128, TH], F16, tag="ot")
                        ps_l = [pp.tile([128, 512], F32, tag="ps", bufs=5,
                                        name=f"ps{tq}")
                                for tq in range(NT_Q)]
                        for ci in range(2):
                            w = wtns[b][ci]
                            mg = megs[b][ci]
                            for tq in range(NT_Q):
                                nc.tensor.matmul(
                                    ps_l[tq], w[:, m0:m0 + 128],
                                    mg[:, t0 + 512 * tq:t0 + 512 * (tq + 1)],
                                    start=(ci == 0), stop=False)
                        for tq in range(NT_Q):
                            nc.tensor.matmul(
                                ps_l[tq],
                                wreps[b][32 * tq:32 * tq + 1, m0:m0 + 128],
                                mgr[32 * tq:32 * tq + 1,
                                    (th * BS + b) * 512:
                                    (th * BS + b + 1) * 512],
                                start=False, stop=True,
                                tile_position=(32 * tq, 0))
                        for tq in range(NT_Q):
                            dst = ot[:, 512 * tq:512 * (tq + 1)]
                            if tq % 2 == 0:
                                nc.vector.tensor_copy(dst, ps_l[tq])
                            else:
                                nc.scalar.activation(out=dst, in_=ps_l[tq],
                                                     func=Copy)
                        store_queue(b, mi).dma_start(
                            out=out_d[b, m0:m0 + 128, t0:t0 + TH], in_=ot)

            # ---- phase 2b: the 14-row chout tail of all samples, packed ----
            # 4 concurrent column-tiles (one per sample) + diagonal K=1 tiles
            om3 = megp.tile([97 + 13, T], F16, tag="om3")
            for tq8 in range(T // 512):
                sl = slice(512 * tq8, 512 * (tq8 + 1))
                ps3 = pp.tile([128, 512], F32, tag="ps", bufs=5)
                for ci in range(2):
                    for b in range(BS):
                        nc.tensor.matmul(
                            ps3[32 * b:32 * b + 14, :],
                            wtns[b][ci][:, 256:CHOUT], megs[b][ci][:, sl],
                            start=(ci == 0), stop=False,
                            tile_position=(0, 32 * b))
                for b in range(BS):
                    nc.tensor.matmul(
                        ps3[32 * b:32 * b + 14, :],
                        wreps[b][32 * b:32 * b + 1, 256:CHOUT],
                        mgr2[32 * b:32 * b + 1, sl],
                        start=False, stop=True,
                        tile_position=(32 * b, 32 * b))
                for b in range(BS):
                    dst = om3[32 * b:32 * b + 14, sl]
                    src = ps3[32 * b:32 * b + 14, :]
                    if b % 2 == 0:
                        nc.vector.tensor_copy(dst, src)
                    else:
                        nc.scalar.activation(out=dst, in_=src, func=Copy)
                if tq8 % NT_Q == NT_Q - 1:
                    th = tq8 // NT_Q
                    for b in range(BS):
                        nc.gpsimd.dma_start(
                            out=out_d[b, 256:CHOUT, th * TH:(th + 1) * TH],
                            in_=om3[32 * b:32 * b + 14, th * TH:(th + 1) * TH])

    nc.compile()
    return nc


def _prep_host_fast(meg, positions, subject_index, heads):
    f32, f16 = np.float32, np.float16
    pos = np.asarray(positions, dtype=f32)
    a = ((pos[:, :, 0] + MARGIN) / WIDTH).astype(f32)
    bc = ((pos[:, :, 1] + MARGIN) / WIDTH).astype(f32)
    fr = np.arange(N_FREQS, dtype=f32)
    fi = np.repeat(fr, N_FREQS)
    fj = np.tile(fr, N_FREQS)

    h = np.asarray(heads, dtype=f32)[
        np.asarray(subject_index).astype(np.int64)]          # [B, 270, 242]
    hT = h.transpose(0, 2, 1)                                # [B, 242, 270]
    hh = np.zeros((B, 128, 2 * CHOUT), dtype=f16)
    hh[:, :NF2, :CHOUT] = hT[:, :NF2, :]                     # cos part
    hh[:, :NF2, CHOUT:] = hT[:, NF2:, :]                     # sin part

    megf = np.asarray(meg, dtype=f32)[:, :C_USED_FAST, :].astype(f16)

    in_maps = []
    for c in range(N_CORES):
        s = slice(c * BS, (c + 1) * BS)
        pos3 = np.zeros((3, PW), dtype=f32)
        pos3[0, :CB] = a[s].reshape(-1)
        pos3[1, :CB] = bc[s].reshape(-1)
        pos3[2, :CB] = 1.0
        pos3[0, CB:CB + NF2] = fi
        pos3[1, CB:CB + NF2] = fj
        pos3[2, CB:CB + NF2] = 0.25                          # cos shift
        pos3[0, CB + NF2:] = fi
        pos3[1, CB + NF2:] = fj
        pos3[2, CB + NF2:] = 0.0                             # sin shift
        in_maps.append(dict(
            meg=np.ascontiguousarray(megf[s]),
            pos3=pos3,
            onesr=np.ones((1, 128), dtype=f32),
            hh=np.ascontiguousarray(hh[s]),
        ))
    return in_maps


def _fast_path_ok(meg, positions, subject_index, heads):
    pos = np.asarray(positions)
    if (np.asarray(meg).shape != (B, C, T) or pos.shape != (B, C, 2)
            or np.asarray(heads).shape[1:] != (CHOUT, 2 * NF2)):
        return False
    invalid = np.all(pos == INVALID, axis=-1)                # [B, C]
    return bool(np.all(~invalid[:, :C_USED_FAST])
                and np.all(invalid[:, C_USED_FAST:]))


# --------------------------------------------------------------------------
# fallback builder (general case: arbitrary invalid-channel masks)
# --------------------------------------------------------------------------

def _c_chunks(c_used):
    out = []
    c0 = 0
    while c0 < c_used:
        out.append((c0, min(128, c_used - c0)))
        c0 += 128
    return out


def _build_bass(c_used, robust_frac=False):
    import concourse.bacc as bacc
    import concourse.mybir as mybir
    import concourse.tile as tile
    import concourse.bass as bass

    F32 = mybir.dt.float32
    F32R = mybir.dt.float32r
    BF16 = mybir.dt.bfloat16
    I32 = mybir.dt.int32
    Sin = mybir.ActivationFunctionType.Sin
    Exp = mybir.ActivationFunctionType.Exp
    Copy = mybir.ActivationFunctionType.Copy
    F16 = mybir.dt.float16

    CC = _c_chunks(c_used)
    NCC = len(CC)
    # a trailing single-channel chunk is handled as one concurrent
    # row-tiled wave across the 4 t-chunks instead of 4 full 512-col passes
    K1_WAVE = CC[-1][1] == 1 and NT_Q == 4
    CCF = CC[:-1] if K1_WAVE else CC          # full chunks
    NF = len(CCF)

    nc = bacc.Bacc("TRN2", target_bir_lowering=False, debug=False,
                   num_devices=N_CORES)

    meg_d = nc.dram_tensor("meg", [BS, C, T], BF16, kind="ExternalInput")
    pa_d = nc.dram_tensor("pa", [BS, C], F32, kind="ExternalInput")
    pb_d = nc.dram_tensor("pb", [BS, C], F32, kind="ExternalInput")
    offs_d = nc.dram_tensor("offs", [BS, C], F32R, kind="ExternalInput")
    hta_d = nc.dram_tensor("hta", [BS, D_A, CHOUT], F32R, kind="ExternalInput")
    htb_d = nc.dram_tensor("htb", [BS, NF2, CHOUT], F32R, kind="ExternalInput")
    fi_d = nc.dram_tensor("fi", [NF2, 1], F32, kind="ExternalInput")
    fj_d = nc.dram_tensor("fj", [NF2, 1], F32, kind="ExternalInput")
    ones_d = nc.dram_tensor("ones", [128, 1], BF16, kind="ExternalInput")
    out_d = nc.dram_tensor("out", [BS, CHOUT, T], F16, kind="ExternalOutput")

    with tile.TileContext(nc) as tc:
        with (
            tc.tile_pool(name="const", bufs=1) as const,
            tc.tile_pool(name="emb1", bufs=2) as emb1,
            tc.tile_pool(name="wsb", bufs=4) as wsb,
            tc.tile_pool(name="persist", bufs=BS) as persist,
            tc.tile_pool(name="megp", bufs=3) as megp,
            tc.tile_pool(name="megp2", bufs=3) as megp2,
            tc.tile_pool(name="outp", bufs=3) as outp,
            tc.tile_pool(name="wps", bufs=1, space="PSUM") as wps,
            tc.tile_pool(name="bps", bufs=6, space="PSUM") as bps,
        ):
            megs_cache = {}

            NTH = T // TH

            def load_megs(b):
                if b in megs_cache:
                    return megs_cache.pop(b)
                megs = []
                for ci, (c0, cs) in enumerate(CCF):
                    pool = megp if cs > 64 else megp2
                    mg = pool.tile([cs, T], BF16, tag=f"mg{ci}")
                    nc.sync.dma_start(out=mg, in_=meg_d[b, c0:c0 + cs, :])
                    megs.append(mg)
                if K1_WAVE:
                    c0 = CC[-1][0]
                    # partition 32q holds t-chunk q of each t-half:
                    # [th0_q | th1_q] along the free dim
                    mgr = megp2.tile([97, NTH * 512], BF16, tag="mgr")
                    src = meg_d[b, c0, :].rearrange(
                        "(h q f) -> q h f", h=NTH, q=NT_Q)
                    dst = mgr[0:97:32, :].rearrange(
                        "q (h f) -> q h f", h=NTH)
                    nc.sync.dma_start(out=dst, in_=src)
                    megs.append(mgr)
                return megs

            def prefetch_megs(b):
                megs_cache[b] = load_megs(b)

            fi = const.tile([NF2, 1], F32, tag="fi")
            fj = const.tile([NF2, 1], F32, tag="fj")
            ones = const.tile([128, 1], BF16, tag="ones")
            nc.sync.dma_start(out=fi, in_=fi_d[:, :])
            nc.sync.dma_start(out=fj, in_=fj_d[:, :])
            nc.sync.dma_start(out=ones, in_=ones_d[:, :])

            # ---- phase 1a: fourier embeddings ---------------------------
            # emitted in two chunks (sample 0, then samples 1..3) so sample
            # 0's weight stage unblocks the PE as early as possible
            embAs = {}

            def emit_emb(b0, nb):
                w = nb * C
                a_rep = emb1.tile([NF2, w], F32, tag="s0")
                b_rep = emb1.tile([NF2, w], F32, tag="s1")
                pa_bcast = bass.AP(tensor=pa_d, offset=b0 * C,
                                   ap=[[0, NF2], [1, w]])
                pb_bcast = bass.AP(tensor=pb_d, offset=b0 * C,
                                   ap=[[0, NF2], [1, w]])
                nc.sync.dma_start(out=a_rep, in_=pa_bcast)
                nc.sync.dma_start(out=b_rep, in_=pb_bcast)

                xs = emb1.tile([NF2, w], F32, tag="s2")
                nc.vector.tensor_scalar_mul(out=xs, in0=a_rep, scalar1=fi)
                xs2 = emb1.tile([NF2, w], F32, tag="s3")
                nc.vector.tensor_scalar_mul(out=xs2, in0=b_rep, scalar1=fj)
                nc.vector.tensor_add(out=xs, in0=xs, in1=xs2)

                embA = emb1.tile([D_A, w], F32R, tag="embA")
                embB = emb1.tile([NF2, w], F32R, tag="embB")

                def reduce_frac(src):
                    ki = emb1.tile([NF2, w], I32, tag="ki")
                    kf = emb1.tile([NF2, w], F32, tag="kf")
                    frac = emb1.tile([NF2, w], F32, tag="fr")
                    # range reduction to [-0.5, 0.5] via f32->int32 cast.
                    # HW rounds to nearest so one stage suffices; CoreSim
                    # truncates, so sim builds add a comparison-based
                    # wraparound stage.
                    nc.vector.tensor_copy(ki, src)
                    nc.vector.tensor_copy(kf, ki)
                    nc.vector.tensor_sub(out=frac, in0=src, in1=kf)
                    if robust_frac:
                        nc.vector.tensor_scalar(
                            out=kf, in0=frac, scalar1=0.5, scalar2=None,
                            op0=mybir.AluOpType.is_gt)
                        nc.vector.tensor_sub(out=frac, in0=frac, in1=kf)
                        nc.vector.tensor_scalar(
                            out=kf, in0=frac, scalar1=-0.5, scalar2=None,
                            op0=mybir.AluOpType.is_lt)
                        nc.vector.tensor_add(out=frac, in0=frac, in1=kf)
                    return frac

                fr1 = reduce_frac(xs)
                nc.scalar.activation(out=embB, in_=fr1, func=Sin,
                                     scale=SCALE_2PI)
                # cos half: cos(2pi x) = sin(2pi (x + 0.25))
                nc.vector.tensor_scalar_add(out=xs2, in0=xs, scalar1=0.25)
                fr2 = reduce_frac(xs2)
                nc.scalar.activation(out=embA[0:NF2, :], in_=fr2, func=Sin,
                                     scale=SCALE_2PI)
                offs_flat = bass.AP(tensor=offs_d, offset=b0 * C,
                                    ap=[[w, 1], [1, w]])
                nc.sync.dma_start(out=embA[NF2:D_A, :], in_=offs_flat)
                for i in range(nb):
                    embAs[b0 + i] = (embA, embB, i * C)

            def emit_weight_stage(b):
                hta = wsb.tile([D_A, CHOUT], F32R, tag="hta")
                htb = wsb.tile([NF2, CHOUT], F32R, tag="htb")
                nc.sync.dma_start(out=hta, in_=hta_d[b, :, :])
                nc.sync.dma_start(out=htb, in_=htb_d[b, :, :])
                embA, embB, co = embAs[b]

                wt = []
                for ci, (c0, cs) in enumerate(CC):
                    ps_s = wps.tile([128, CHOUT], F32, tag="ps_s")
                    nc.tensor.matmul(ps_s[0:cs, :],
                                     embA[:, co + c0:co + c0 + cs], hta,
                                     start=True, stop=False)
                    nc.tensor.matmul(ps_s[0:cs, :],
                                     embB[:, co + c0:co + c0 + cs], htb,
                                     start=False, stop=True)
                    if K1_WAVE and ci == NCC - 1:
                        # single-channel chunk: replicate exp(weights) at
                        # partitions 0/32/64/96 for the row-tiled wave
                        wrep = persist.tile([97, CHOUT], BF16, tag="wrep")
                        for q in range(NT_Q):
                            nc.scalar.activation(
                                out=wrep[32 * q:32 * q + 1, :],
                                in_=ps_s[0:1, :], func=Exp)
                        wt.append(wrep)
                    else:
                        w_un = persist.tile([128, CHOUT], BF16,
                                            tag=f"w_un{ci}")
                        nc.scalar.activation(out=w_un[0:cs, :],
                                             in_=ps_s[0:cs, :], func=Exp)
                        wt.append(w_un)

                invs = []
                for mi, (m0, ms) in enumerate(M_CHUNKS):
                    ps_sum = wps.tile([128, 1], F32, tag="ps_sum")
                    for ci, (c0, cs) in enumerate(CC):
                        nc.tensor.matmul(ps_sum[0:ms, :],
                                         wt[ci][0:cs, m0:m0 + ms],
                                         ones[0:cs, :],
                                         start=(ci == 0), stop=(ci == NCC - 1))
                    inv = persist.tile([128, 1], F32, tag=f"inv{mi}")
                    nc.vector.reciprocal(out=inv[0:ms, :], in_=ps_sum[0:ms, :])
                    invs.append(inv)
                return wt, invs

            wts, invss = [None] * BS, [None] * BS
            emit_emb(0, 1)
            wts[0], invss[0] = emit_weight_stage(0)
            prefetch_megs(0)
            emit_emb(1, BS - 1)
            for b in range(1, BS):
                wts[b], invss[b] = emit_weight_stage(b)

            # ---- phase 2: big matmuls, PE back-to-back -----------------
            for b in range(BS):
                wt, invs = wts[b], invss[b]
                megs = load_megs(b)
                for th in range(T // TH):
                    t0 = th * TH
                    for mi, (m0, ms) in enumerate(M_CHUNKS):
                        ot = outp.tile([ms, TH], F16, tag=f"ot{mi}")
                        ps_list = []
                        for tq in range(NT_Q):
                            ps_o = bps.tile([128, 512], F32, tag="ps_o")
                            ps_list.append(ps_o)
                            for ci, (c0, cs) in enumerate(CCF):
                                nc.tensor.matmul(
                                    ps_o[0:ms, :],
                                    wt[ci][0:cs, m0:m0 + ms],
                                    megs[ci][:, t0 + tq * 512:
                                             t0 + (tq + 1) * 512],
                                    start=(ci == 0),
                                    stop=(not K1_WAVE and ci == NF - 1))
                        if K1_WAVE:
                            # single-channel contribution: 4 concurrent
                            # row-tiled K=1 matmuls (one per t-chunk)
                            for tq in range(NT_Q):
                                nc.tensor.matmul(
                                    ps_list[tq][0:ms, :],
                                    wt[-1][32 * tq:32 * tq + 1, m0:m0 + ms],
                                    megs[-1][32 * tq:32 * tq + 1,
                                             th * 512:(th + 1) * 512],
                                    start=False, stop=True,
                                    tile_position=(32 * tq, 0))
                        for tq in range(NT_Q):
                            # scaled psum->sbuf copy; alternate DVE/ACT so
                            # neither engine becomes the bottleneck
                            if tq % 2 == 0:
                                nc.vector.tensor_scalar_mul(
                                    out=ot[:, tq * 512:(tq + 1) * 512],
                                    in0=ps_list[tq][0:ms, :],
                                    scalar1=invs[mi][0:ms, :])
                            else:
                                nc.scalar.activation(
                                    out=ot[:, tq * 512:(tq + 1) * 512],
                                    in_=ps_list[tq][0:ms, :], func=Copy,
                                    scale=invs[mi][0:ms, :])
                        if b == BS - 1 and th == T // TH - 1:
                            nc.gpsimd.dma_start(
                                out=out_d[b, m0:m0 + ms, t0:t0 + TH // 2],
                                in_=ot[:, 0:TH // 2])
                            nc.gpsimd.dma_start(
                                out=out_d[b, m0:m0 + ms,
                                          t0 + TH // 2:t0 + TH],
                                in_=ot[:, TH // 2:TH])
                        else:
                            nc.gpsimd.dma_start(
                                out=out_d[b, m0:m0 + ms, t0:t0 + TH], in_=ot)

    nc.compile()
    return nc


def _get_nc(key):
    if key not in _NC_CACHE:
        if key == "fast":
            _NC_CACHE[key] = _build_fast()
        else:
            _NC_CACHE[key] = _build_bass(key)
    return _NC_CACHE[key]


def _prep_host(meg, positions, subject_index, heads):
    """Build the 8 per-core input maps + pick the channel prefix length."""
    f32 = np.float32
    pos = np.asarray(positions, dtype=f32)
    a = ((pos[:, :, 0] + MARGIN) / WIDTH).astype(f32)           # [B, C]
    bcoord = ((pos[:, :, 1] + MARGIN) / WIDTH).astype(f32)      # [B, C]
    invalid = np.all(pos == INVALID, axis=-1)                   # [B, C]
    offs = np.where(invalid, f32(NEG_INF), f32(0.0)).astype(f32)

    # channels invalid in EVERY sample get weight exactly 0 (exp(-1e9)==0)
    # -> their meg data is never needed; use the valid prefix length
    valid_any = ~np.all(invalid, axis=0)                        # [C]
    c_used = int(np.max(np.nonzero(valid_any)[0])) + 1 if valid_any.any() else C

    h = np.asarray(heads, dtype=f32)[np.asarray(subject_index).astype(np.int64)]
    hT = np.ascontiguousarray(h.transpose(0, 2, 1))             # [B, 242, O]
    hta = np.concatenate(
        [hT[:, :NF2, :], np.ones((B, 1, CHOUT), dtype=f32)], axis=1)
    htb = np.ascontiguousarray(hT[:, NF2:, :])

    fr = np.arange(N_FREQS, dtype=f32)
    fi = np.repeat(fr, N_FREQS).reshape(NF2, 1)
    fj = np.tile(fr, N_FREQS).reshape(NF2, 1)
    import ml_dtypes as _mld
    ones = np.ones((128, 1), dtype=_mld.bfloat16)

    import ml_dtypes
    megf = np.asarray(meg, dtype=f32).astype(ml_dtypes.bfloat16)
    in_maps = []
    for c in range(N_CORES):
        s = slice(c * BS, (c + 1) * BS)
        in_maps.append(dict(
            meg=np.ascontiguousarray(megf[s]),
            pa=np.ascontiguousarray(a[s]),
            pb=np.ascontiguousarray(bcoord[s]),
            offs=np.ascontiguousarray(offs[s]),
            hta=np.ascontiguousarray(hta[s]),
            htb=np.ascontiguousarray(htb[s]),
            fi=fi, fj=fj, ones=ones,
        ))
    return in_maps, c_used


def kernel(meg, positions, subject_index, heads, _trace=False):
    from concourse.bass_utils import run_bass_kernel_spmd

    if _fast_path_ok(meg, positions, subject_index, heads):
        in_maps = _prep_host_fast(meg, positions, subject_index, heads)
        nc = _get_nc("fast")
    else:
        in_maps, c_used = _prep_host(meg, positions, subject_index, heads)
        nc = _get_nc(c_used)
    res = run_bass_kernel_spmd(nc, in_maps, core_ids=list(range(N_CORES)),
                               trace=_trace)
    out = np.concatenate([r["out"] for r in res.results], axis=0)
    if _trace:
        kernel.last_exec_time_ns = res.exec_time_ns
        kernel.last_results = res
    return out.astype(np.float32)


# revision 17
# speedup vs baseline: 1.1556x; 1.1556x over previous
"""TRN2 Bass kernel for nn_BrainModule (sparse_attention).

Computation (per sample b):
  emb[c,d]   = fourier embedding of positions[b,c]          (d = 242)
  scores[o,c]= heads[subj[b]][o,:] . emb[c,:] + offset[c]   (offset = -1e9 on
                                                             invalid channels)
  w[o,c]     = softmax_c(scores)
  out[o,t]   = sum_c w[o,c] * meg[b,c,t]

Data-parallel over batch B=32 across 8 cores (4 samples each).

Fast path (taken when the invalid channels are exactly the common suffix
257..272, which the module always produces):
  - Embeddings via a K=3 PE outer-product matmul (stationary [fi, fj, shift]
    columns x moving [pa, pb, 1] rows), int-cast range reduction, ACT Sin.
    No 121-way broadcast DMAs.
  - Scores in fp16 with emb/heads K-padded to 128 so FWL (fast weight load)
    engages; softmax sums via a ones-stationary matmul -> [1, 270], then a
    K=1 replicate matmul + DVE reciprocal; weights pre-scaled by 1/sum on
    DVE so the big-matmul PSUM->SBUF copies are plain copies.
  - The big einsum in fp16 (more mantissa than bf16 at the same byte cost),
    output chout rows 0..255 as two 128-row chunks per sample; the awkward
    14-row tail chunk (256..270) of all 4 samples is packed into 4
    concurrent PE column-tiles (tile_position=(0,32b)), and the K=1 channel
    rides diagonal (32b,32b) tiles -- ~3x less PE time for that chunk.
  - DMA split across three queues: sync HWDGE (meg chunk 0), scalar HWDGE
    (heads + meg chunk 1), gpsimd SWDGE (most stores); late stores spill to
    the by-then-idle HWDGE queues.
"""
import numpy as np

B, C, T = 32, 273, 4096
CHOUT = 270
N_FREQS = 11
NF2 = N_FREQS * N_FREQS          # 121
D_A = NF2 + 1                    # cos half + offset/ones row (fallback path)
MARGIN = 0.2
WIDTH = 1.0 + 2.0 * MARGIN
INVALID = -0.1
NEG_INF = -1e9
N_CORES = 8
BS = B // N_CORES                # samples per core
CB = BS * C                      # batched embedding width (1092)
C_USED_FAST = 257                # fast-path channel prefix
CBU = BS * C_USED_FAST           # packed used-channel width (1028)
PW = CBU + 2 * NF2               # pos3 packed width (1270)
TWO_PI = float(2.0 * np.pi)
# largest f32 <= 2*pi, so |frac| = 0.5 never maps beyond pi
SCALE_2PI = float(np.nextafter(np.float32(2.0 * np.pi), np.float32(0.0)))

M_CHUNKS = [(0, 128), (128, 128), (256, CHOUT - 256)]  # partition chunks of O
TH = 2048                                              # meg/out tile t width
NT_Q = TH // 512                                       # 512-wide psum tiles

_NC_CACHE = {}


# --------------------------------------------------------------------------
# fast-path builder
# --------------------------------------------------------------------------

def _build_fast():
    import concourse.bacc as bacc
    import concourse.mybir as mybir
    import concourse.tile as tile

    F32 = mybir.dt.float32
    F32R = mybir.dt.float32r
    F16 = mybir.dt.float16
    I32 = mybir.dt.int32
    Sin = mybir.ActivationFunctionType.Sin
    Exp = mybir.ActivationFunctionType.Exp
    Copy = mybir.ActivationFunctionType.Copy

    NTH = T // TH                # 2

    nc = bacc.Bacc("TRN2", target_bir_lowering=False, debug=False,
                   num_devices=N_CORES)

    meg_d = nc.dram_tensor("meg", [BS, C_USED_FAST, T], F16,
                           kind="ExternalInput")
    pos3_d = nc.dram_tensor("pos3", [3, PW], F32R, kind="ExternalInput")
    onesr_d = nc.dram_tensor("onesr", [1, 128], F32R, kind="ExternalInput")
    hh_d = nc.dram_tensor("hh", [BS, 128, 2 * CHOUT], F16,
                          kind="ExternalInput")
    out_d = nc.dram_tensor("out", [BS, CHOUT, T], F16, kind="ExternalOutput")

    with tile.TileContext(nc) as tc:
        with (
            tc.tile_pool(name="const", bufs=1) as const,
            tc.tile_pool(name="embsb", bufs=1) as embsb,
            tc.tile_pool(name="embw", bufs=2) as embw,
            tc.tile_pool(name="wsb", bufs=1) as wsb,
            tc.tile_pool(name="wraw", bufs=2) as wraw,
            tc.tile_pool(name="persist", bufs=1) as persist,
            tc.tile_pool(name="megp", bufs=1) as megp,
            tc.tile_pool(name="outp", bufs=3) as outp,
            tc.tile_pool(name="pp", bufs=1, space="PSUM") as pp,
        ):
            # ---- input DMAs ------------------------------------------------
            # consolidated transfers; each meg sample's two 128-chunks split
            # across the two HWDGE queues so sample b lands in ~half the time.
            # Emission interleaves per-sample loads with the compute stages so
            # neither engine front-loads a long run of DMA-issue instructions.
            pos3 = const.tile([3, PW], F32R, tag="pos3")
            nc.sync.dma_start(out=pos3, in_=pos3_d[:, :])

            ones_col = const.tile([128, 1], F16, tag="ones_col")
            nc.gpsimd.memset(ones_col, 1.0)
            ones_row = const.tile([1, 128], F32R, tag="ones_row")
            nc.sync.dma_start(out=ones_row, in_=onesr_d[:, :])

            hhs, megs, mg_tiles = [], [], []
            for b in range(BS):
                hhs.append(wsb.tile([128, 2 * CHOUT], F16, tag=f"hh_{b}",
                                    name=f"hh{b}"))
                mg = megp.tile([128, 2 * T], F16, tag=f"mg_{b}",
                               name=f"mg{b}")
                mg_tiles.append(mg)
                megs.append([mg[:, 0:T], mg[:, T:2 * T]])

            def load_sample(b):
                nc.scalar.dma_start(out=hhs[b], in_=hh_d[b, :, :])
                nc.sync.dma_start(out=mg_tiles[b][:, 0:T],
                                  in_=meg_d[b, 0:128, :])
                nc.scalar.dma_start(out=mg_tiles[b][:, T:2 * T],
                                    in_=meg_d[b, 128:256, :])

            load_sample(0)
            # k1 channel, tq-quadrant layout for the per-sample K1 wave:
            # partition 32q holds [th, b, 512] for t-chunk q (th-major so
            # each DMA stays within the 3-dim AP limit)
            mgr = megp.tile([97, NTH * BS * 512], F16, tag="mgr")
            for th in range(NTH):
                src = meg_d[:, 256, th * TH:(th + 1) * TH].rearrange(
                    "b (q f) -> q b f", q=NT_Q)
                dst = mgr[0:97:32, th * BS * 512:(th + 1) * BS * 512
                          ].rearrange("q (b f) -> q b f", b=BS)
                nc.sync.dma_start(out=dst, in_=src)
            # k1 channel, sample-quadrant layout for the m3 diagonal
            mgr2 = megp.tile([97, T], F16, tag="mgr2")
            nc.sync.dma_start(out=mgr2[0:97:32, :], in_=meg_d[:, 256, :])

            # ---- embeddings -------------------------------------------------
            # embA = cos half, embB = sin half, used channels of all samples
            # packed along the free dim; rows 121..127 zero so K=128 score
            # matmuls get FWL
            embA = embsb.tile([128, CBU], F16, tag="embA")
            embB = embsb.tile([128, CBU], F16, tag="embB")
            # memset partition start must be 32-aligned; rows 96..120 are
            # overwritten by the ACT sin writes afterwards (WAW-ordered)
            nc.gpsimd.memset(embA[96:128, :], 0.0)
            nc.gpsimd.memset(embB[96:128, :], 0.0)

            fifi = [pos3[:, CBU:CBU + NF2],            # cos: [fi, fj, 0.25]
                    pos3[:, CBU + NF2:CBU + 2 * NF2]]  # sin: [fi, fj, 0]
            embT = [embA, embB]

            def emb_stage(half, w0, wn):
                ps_e = pp.tile([NF2, wn], F32, tag="ws", bufs=2, name="ps_e")
                nc.tensor.matmul(ps_e, fifi[half], pos3[:, w0:w0 + wn],
                                 start=True, stop=True)
                # range reduction to [-0.5, 0.5]: HW f32->i32 cast rounds to
                # nearest, so frac = x - round(x)
                ki = embw.tile([NF2, wn], I32, tag="ki")
                kf = embw.tile([NF2, wn], F32, tag="kf")
                fr = embw.tile([NF2, wn], F32, tag="fr")
                nc.vector.tensor_copy(ki, ps_e)
                nc.vector.tensor_copy(kf, ki)
                nc.vector.tensor_sub(out=fr, in0=ps_e, in1=kf)
                nc.scalar.activation(out=embT[half][0:NF2, w0:w0 + wn],
                                     in_=fr, func=Sin, scale=SCALE_2PI)

            # ---- weight stage: scores -> softmax weights, pre-normalized ---
            wtns, wreps = [], []

            def weight_stage(b):
                co = b * C_USED_FAST
                hta = hhs[b][:, 0:CHOUT]
                htb = hhs[b][:, CHOUT:2 * CHOUT]
                ps_s0 = pp.tile([128, CHOUT], F32, tag="ws", bufs=2)
                nc.tensor.matmul(ps_s0, embA[:, co:co + 128], hta,
                                 start=True, stop=False)
                nc.tensor.matmul(ps_s0, embB[:, co:co + 128], htb,
                                 start=False, stop=True)
                ps_s1 = pp.tile([128, CHOUT], F32, tag="ws", bufs=2)
                nc.tensor.matmul(ps_s1, embA[:, co + 128:co + 256], hta,
                                 start=True, stop=False)
                nc.tensor.matmul(ps_s1, embB[:, co + 128:co + 256], htb,
                                 start=False, stop=True)
                ps_s2 = pp.tile([1, CHOUT], F32, tag="ws", bufs=2)
                nc.tensor.matmul(ps_s2, embA[:, co + 256:co + 257], hta,
                                 start=True, stop=False)
                nc.tensor.matmul(ps_s2, embB[:, co + 256:co + 257], htb,
                                 start=False, stop=True)

                wt0 = wraw.tile([128, CHOUT], F16, tag="wt0")
                wt1 = wraw.tile([128, CHOUT], F16, tag="wt1")
                ek1 = wraw.tile([1, CHOUT], F32, tag="ek1")
                nc.scalar.activation(out=wt0, in_=ps_s0, func=Exp)
                nc.scalar.activation(out=wt1, in_=ps_s1, func=Exp)
                nc.scalar.activation(out=ek1, in_=ps_s2, func=Exp)

                ps_sum = pp.tile([1, CHOUT], F32, tag="ws", bufs=2)
                nc.tensor.matmul(ps_sum, ones_col, wt0,
                                 start=True, stop=False)
                nc.tensor.matmul(ps_sum, ones_col, wt1,
                                 start=False, stop=True)
                sums = wraw.tile([1, CHOUT], F32R, tag="sums")
                nc.vector.tensor_add(out=sums, in0=ps_sum, in1=ek1)
                ps_rep = pp.tile([128, CHOUT], F32, tag="ws", bufs=2)
                nc.tensor.matmul(ps_rep, ones_row, sums,
                                 start=True, stop=True)
                invrep = wraw.tile([128, CHOUT], F16, tag="invrep")
                with nc.allow_low_precision(
                        reason="1/sum in f16: 5e-4 rel vs 2e-2 budget"):
                    nc.vector.reciprocal(out=invrep, in_=ps_rep)

                wtn0 = persist.tile([128, CHOUT], F16, tag=f"wtn0_{b}")
                wtn1 = persist.tile([128, CHOUT], F16, tag=f"wtn1_{b}")
                wrep = persist.tile([97, CHOUT], F16, tag=f"wrep_{b}")
                nc.vector.tensor_mul(out=wtn0, in0=wt0, in1=invrep)
                nc.vector.tensor_mul(out=wtn1, in0=wt1, in1=invrep)
                wk1 = wraw.tile([1, CHOUT], F32R, tag="wk1")
                nc.vector.tensor_mul(out=wk1, in0=ek1, in1=invrep[0:1, :])
                # replicate the k1-channel weights to partitions 0..96 via a
                # K=1 matmul (keeps ACT on a pure Sin->Exp->Copy table order)
                ps_rp2 = pp.tile([128, CHOUT], F32, tag="ws", bufs=2)
                nc.tensor.matmul(ps_rp2, ones_row, wk1,
                                 start=True, stop=True)
                nc.vector.tensor_copy(wrep, ps_rp2[0:97, :])
                wtns.append([wtn0, wtn1])
                wreps.append(wrep)

            # sample 0's embedding + weight stage first so its big matmuls
            # start asap; remaining samples' loads interleave with compute.
            # fp32r matmul ISA restriction: moving width must be even.
            W_A = C_USED_FAST + 1                      # 258
            W_B = (CBU - W_A) // 2                     # 385 -> make even: 384/386
            emb_stage(0, 0, W_A)
            emb_stage(1, 0, W_A)
            weight_stage(0)
            load_sample(1)
            emb_stage(0, W_A, 386)
            emb_stage(0, W_A + 386, CBU - W_A - 386)
            emb_stage(1, W_A, 386)
            emb_stage(1, W_A + 386, CBU - W_A - 386)
            weight_stage(1)
            load_sample(2)
            weight_stage(2)
            load_sample(3)
            weight_stage(3)

            # ---- phase 2a: per-sample big matmuls (chout rows 0..255) ------
            def store_queue(b, mi):
                # gpsimd early; late stores spill to the by-then-idle HWDGE
                # queues (their load FIFOs have drained)
                if b <= 1:
                    return nc.gpsimd
                if b == 2:
                    return nc.gpsimd if mi == 0 else nc.sync
                return nc.scalar if mi == 0 else nc.sync

            for b in range(BS):
                for th in range(NTH):
                    t0 = th * TH
                    for mi in range(2):
                        m0 = mi * 128
                        ot = outp.tile([128, TH], F16, tag="ot")
                        ps_l = [pp.tile([128, 512], F32, tag="ps", bufs=6,
                                        name=f"ps{tq}")
                                for tq in range(NT_Q)]
                        for ci in range(2):
                            w = wtns[b][ci]
                            mg = megs[b][ci]
                            for tq in range(NT_Q):
                                nc.tensor.matmul(
                                    ps_l[tq], w[:, m0:m0 + 128],
                                    mg[:, t0 + 512 * tq:t0 + 512 * (tq + 1)],
                                    start=(ci == 0), stop=False)
                        for tq in range(NT_Q):
                            nc.tensor.matmul(
                                ps_l[tq],
                                wreps[b][32 * tq:32 * tq + 1, m0:m0 + 128],
                                mgr[32 * tq:32 * tq + 1,
                                    (th * BS + b) * 512:
                                    (th * BS + b + 1) * 512],
                                start=False, stop=True,
                                tile_position=(32 * tq, 0))
                        for tq in range(NT_Q):
                            dst = ot[:, 512 * tq:512 * (tq + 1)]
                            if tq % 2 == 0:
                                nc.vector.tensor_copy(dst, ps_l[tq])
                            else:
                                nc.scalar.activation(out=dst, in_=ps_l[tq],
                                                     func=Copy)
                        store_queue(b, mi).dma_start(
                            out=out_d[b, m0:m0 + 128, t0:t0 + TH], in_=ot)

            # ---- phase 2b: the 14-row chout tail of all samples, packed ----
            # 4 concurrent column-tiles (one per sample) + diagonal K=1 tiles
            om3 = megp.tile([97 + 13, T], F16, tag="om3")
            for tq8 in range(T // 512):
                sl = slice(512 * tq8, 512 * (tq8 + 1))
                ps3 = pp.tile([128, 512], F32, tag="ps", bufs=6)
                for ci in range(2):
                    for b in range(BS):
                        nc.tensor.matmul(
                            ps3[32 * b:32 * b + 14, :],
                            wtns[b][ci][:, 256:CHOUT], megs[b][ci][:, sl],
                            start=(ci == 0), stop=False,
                            tile_position=(0, 32 * b))
                for b in range(BS):
                    nc.tensor.matmul(
                        ps3[32 * b:32 * b + 14, :],
                        wreps[b][32 * b:32 * b + 1, 256:CHOUT],
                        mgr2[32 * b:32 * b + 1, sl],
                        start=False, stop=True,
                        tile_position=(32 * b, 32 * b))
                for b in range(BS):
                    dst = om3[32 * b:32 * b + 14, sl]
                    src = ps3[32 * b:32 * b + 14, :]
                    if b % 2 == 0:
                        nc.vector.tensor_copy(dst, src)
                    else:
                        nc.scalar.activation(out=dst, in_=src, func=Copy)
                if tq8 % NT_Q == NT_Q - 1:
                    th = tq8 // NT_Q
                    for b in range(BS):
                        nc.gpsimd.dma_start(
                            out=out_d[b, 256:CHOUT, th * TH:(th + 1) * TH],
                            in_=om3[32 * b:32 * b + 14, th * TH:(th + 1) * TH])

    nc.compile()
    return nc


def _prep_host_fast(meg, positions, subject_index, heads):
    f32, f16 = np.float32, np.float16
    pos = np.asarray(positions, dtype=f32)
    a = ((pos[:, :, 0] + MARGIN) / WIDTH).astype(f32)
    bc = ((pos[:, :, 1] + MARGIN) / WIDTH).astype(f32)
    fr = np.arange(N_FREQS, dtype=f32)
    fi = np.repeat(fr, N_FREQS)
    fj = np.tile(fr, N_FREQS)

    h = np.asarray(heads, dtype=f32)[
        np.asarray(subject_index).astype(np.int64)]          # [B, 270, 242]
    hT = h.transpose(0, 2, 1)                                # [B, 242, 270]
    hh = np.zeros((B, 128, 2 * CHOUT), dtype=f16)
    hh[:, :NF2, :CHOUT] = hT[:, :NF2, :]                     # cos part
    hh[:, :NF2, CHOUT:] = hT[:, NF2:, :]                     # sin part

    megf = np.asarray(meg, dtype=f32)[:, :C_USED_FAST, :].astype(f16)

    in_maps = []
    for c in range(N_CORES):
        s = slice(c * BS, (c + 1) * BS)
        pos3 = np.zeros((3, PW), dtype=f32)
        pos3[0, :CBU] = a[s, :C_USED_FAST].reshape(-1)
        pos3[1, :CBU] = bc[s, :C_USED_FAST].reshape(-1)
        pos3[2, :CBU] = 1.0
        pos3[0, CBU:CBU + NF2] = fi
        pos3[1, CBU:CBU + NF2] = fj
        pos3[2, CBU:CBU + NF2] = 0.25                        # cos shift
        pos3[0, CBU + NF2:] = fi
        pos3[1, CBU + NF2:] = fj
        pos3[2, CBU + NF2:] = 0.0                            # sin shift
        in_maps.append(dict(
            meg=np.ascontiguousarray(megf[s]),
            pos3=pos3,
            onesr=np.ones((1, 128), dtype=f32),
            hh=np.ascontiguousarray(hh[s]),
        ))
    return in_maps


def _fast_path_ok(meg, positions, subject_index, heads):
    pos = np.asarray(positions)
    if (np.asarray(meg).shape != (B, C, T) or pos.shape != (B, C, 2)
            or np.asarray(heads).shape[1:] != (CHOUT, 2 * NF2)):
        return False
    invalid = np.all(pos == INVALID, axis=-1)                # [B, C]
    return bool(np.all(~invalid[:, :C_USED_FAST])
                and np.all(invalid[:, C_USED_FAST:]))


# --------------------------------------------------------------------------
# fallback builder (general case: arbitrary invalid-channel masks)
# --------------------------------------------------------------------------

def _c_chunks(c_used):
    out = []
    c0 = 0
    while c0 < c_used:
        out.append((c0, min(128, c_used - c0)))
        c0 += 128
    return out


def _build_bass(c_used, robust_frac=False):
    import concourse.bacc as bacc
    import concourse.mybir as mybir
    import concourse.tile as tile
    import concourse.bass as bass

    F32 = mybir.dt.float32
    F32R = mybir.dt.float32r
    BF16 = mybir.dt.bfloat16
    I32 = mybir.dt.int32
    Sin = mybir.ActivationFunctionType.Sin
    Exp = mybir.ActivationFunctionType.Exp
    Copy = mybir.ActivationFunctionType.Copy
    F16 = mybir.dt.float16

    CC = _c_chunks(c_used)
    NCC = len(CC)
    # a trailing single-channel chunk is handled as one concurrent
    # row-tiled wave across the 4 t-chunks instead of 4 full 512-col passes
    K1_WAVE = CC[-1][1] == 1 and NT_Q == 4
    CCF = CC[:-1] if K1_WAVE else CC          # full chunks
    NF = len(CCF)

    nc = bacc.Bacc("TRN2", target_bir_lowering=False, debug=False,
                   num_devices=N_CORES)

    meg_d = nc.dram_tensor("meg", [BS, C, T], BF16, kind="ExternalInput")
    pa_d = nc.dram_tensor("pa", [BS, C], F32, kind="ExternalInput")
    pb_d = nc.dram_tensor("pb", [BS, C], F32, kind="ExternalInput")
    offs_d = nc.dram_tensor("offs", [BS, C], F32R, kind="ExternalInput")
    hta_d = nc.dram_tensor("hta", [BS, D_A, CHOUT], F32R, kind="ExternalInput")
    htb_d = nc.dram_tensor("htb", [BS, NF2, CHOUT], F32R, kind="ExternalInput")
    fi_d = nc.dram_tensor("fi", [NF2, 1], F32, kind="ExternalInput")
    fj_d = nc.dram_tensor("fj", [NF2, 1], F32, kind="ExternalInput")
    ones_d = nc.dram_tensor("ones", [128, 1], BF16, kind="ExternalInput")
    out_d = nc.dram_tensor("out", [BS, CHOUT, T], F16, kind="ExternalOutput")

    with tile.TileContext(nc) as tc:
        with (
            tc.tile_pool(name="const", bufs=1) as const,
            tc.tile_pool(name="emb1", bufs=2) as emb1,
            tc.tile_pool(name="wsb", bufs=4) as wsb,
            tc.tile_pool(name="persist", bufs=BS) as persist,
            tc.tile_pool(name="megp", bufs=3) as megp,
            tc.tile_pool(name="megp2", bufs=3) as megp2,
            tc.tile_pool(name="outp", bufs=3) as outp,
            tc.tile_pool(name="wps", bufs=1, space="PSUM") as wps,
            tc.tile_pool(name="bps", bufs=6, space="PSUM") as bps,
        ):
            megs_cache = {}

            NTH = T // TH

            def load_megs(b):
                if b in megs_cache:
                    return megs_cache.pop(b)
                megs = []
                for ci, (c0, cs) in enumerate(CCF):
                    pool = megp if cs > 64 else megp2
                    mg = pool.tile([cs, T], BF16, tag=f"mg{ci}")
                    nc.sync.dma_start(out=mg, in_=meg_d[b, c0:c0 + cs, :])
                    megs.append(mg)
                if K1_WAVE:
                    c0 = CC[-1][0]
                    # partition 32q holds t-chunk q of each t-half:
                    # [th0_q | th1_q] along the free dim
                    mgr = megp2.tile([97, NTH * 512], BF16, tag="mgr")
                    src = meg_d[b, c0, :].rearrange(
                        "(h q f) -> q h f", h=NTH, q=NT_Q)
                    dst = mgr[0:97:32, :].rearrange(
                        "q (h f) -> q h f", h=NTH)
                    nc.sync.dma_start(out=dst, in_=src)
                    megs.append(mgr)
                return megs

            def prefetch_megs(b):
                megs_cache[b] = load_megs(b)

            fi = const.tile([NF2, 1], F32, tag="fi")
            fj = const.tile([NF2, 1], F32, tag="fj")
            ones = const.tile([128, 1], BF16, tag="ones")
            nc.sync.dma_start(out=fi, in_=fi_d[:, :])
            nc.sync.dma_start(out=fj, in_=fj_d[:, :])
            nc.sync.dma_start(out=ones, in_=ones_d[:, :])

            # ---- phase 1a: fourier embeddings ---------------------------
            # emitted in two chunks (sample 0, then samples 1..3) so sample
            # 0's weight stage unblocks the PE as early as possible
            embAs = {}

            def emit_emb(b0, nb):
                w = nb * C
                a_rep = emb1.tile([NF2, w], F32, tag="s0")
                b_rep = emb1.tile([NF2, w], F32, tag="s1")
                pa_bcast = bass.AP(tensor=pa_d, offset=b0 * C,
                                   ap=[[0, NF2], [1, w]])
                pb_bcast = bass.AP(tensor=pb_d, offset=b0 * C,
                                   ap=[[0, NF2], [1, w]])
                nc.sync.dma_start(out=a_rep, in_=pa_bcast)
                nc.sync.dma_start(out=b_rep, in_=pb_bcast)

                xs = emb1.tile([NF2, w], F32, tag="s2")
                nc.vector.tensor_scalar_mul(out=xs, in0=a_rep, scalar1=fi)
                xs2 = emb1.tile([NF2, w], F32, tag="s3")
                nc.vector.tensor_scalar_mul(out=xs2, in0=b_rep, scalar1=fj)
                nc.vector.tensor_add(out=xs, in0=xs, in1=xs2)

                embA = emb1.tile([D_A, w], F32R, tag="embA")
                embB = emb1.tile([NF2, w], F32R, tag="embB")

                def reduce_frac(src):
                    ki = emb1.tile([NF2, w], I32, tag="ki")
                    kf = emb1.tile([NF2, w], F32, tag="kf")
                    frac = emb1.tile([NF2, w], F32, tag="fr")
                    # range reduction to [-0.5, 0.5] via f32->int32 cast.
                    # HW rounds to nearest so one stage suffices; CoreSim
                    # truncates, so sim builds add a comparison-based
                    # wraparound stage.
                    nc.vector.tensor_copy(ki, src)
                    nc.vector.tensor_copy(kf, ki)
                    nc.vector.tensor_sub(out=frac, in0=src, in1=kf)
                    if robust_frac:
                        nc.vector.tensor_scalar(
                            out=kf, in0=frac, scalar1=0.5, scalar2=None,
                            op0=mybir.AluOpType.is_gt)
                        nc.vector.tensor_sub(out=frac, in0=frac, in1=kf)
                        nc.vector.tensor_scalar(
                            out=kf, in0=frac, scalar1=-0.5, scalar2=None,
                            op0=mybir.AluOpType.is_lt)
                        nc.vector.tensor_add(out=frac, in0=frac, in1=kf)
                    return frac

                fr1 = reduce_frac(xs)
                nc.scalar.activation(out=embB, in_=fr1, func=Sin,
                                     scale=SCALE_2PI)
                # cos half: cos(2pi x) = sin(2pi (x + 0.25))
                nc.vector.tensor_scalar_add(out=xs2, in0=xs, scalar1=0.25)
                fr2 = reduce_frac(xs2)
                nc.scalar.activation(out=embA[0:NF2, :], in_=fr2, func=Sin,
                                     scale=SCALE_2PI)
                offs_flat = bass.AP(tensor=offs_d, offset=b0 * C,
                                    ap=[[w, 1], [1, w]])
                nc.sync.dma_start(out=embA[NF2:D_A, :], in_=offs_flat)
                for i in range(nb):
                    embAs[b0 + i] = (embA, embB, i * C)

            def emit_weight_stage(b):
                hta = wsb.tile([D_A, CHOUT], F32R, tag="hta")
                htb = wsb.tile([NF2, CHOUT], F32R, tag="htb")
                nc.sync.dma_start(out=hta, in_=hta_d[b, :, :])
                nc.sync.dma_start(out=htb, in_=htb_d[b, :, :])
                embA, embB, co = embAs[b]

                wt = []
                for ci, (c0, cs) in enumerate(CC):
                    ps_s = wps.tile([128, CHOUT], F32, tag="ps_s")
                    nc.tensor.matmul(ps_s[0:cs, :],
                                     embA[:, co + c0:co + c0 + cs], hta,
                                     start=True, stop=False)
                    nc.tensor.matmul(ps_s[0:cs, :],
                                     embB[:, co + c0:co + c0 + cs], htb,
                                     start=False, stop=True)
                    if K1_WAVE and ci == NCC - 1:
                        # single-channel chunk: replicate exp(weights) at
                        # partitions 0/32/64/96 for the row-tiled wave
                        wrep = persist.tile([97, CHOUT], BF16, tag="wrep")
                        for q in range(NT_Q):
                            nc.scalar.activation(
                                out=wrep[32 * q:32 * q + 1, :],
                                in_=ps_s[0:1, :], func=Exp)
                        wt.append(wrep)
                    else:
                        w_un = persist.tile([128, CHOUT], BF16,
                                            tag=f"w_un{ci}")
                        nc.scalar.activation(out=w_un[0:cs, :],
                                             in_=ps_s[0:cs, :], func=Exp)
                        wt.append(w_un)

                invs = []
                for mi, (m0, ms) in enumerate(M_CHUNKS):
                    ps_sum = wps.tile([128, 1], F32, tag="ps_sum")
                    for ci, (c0, cs) in enumerate(CC):
                        nc.tensor.matmul(ps_sum[0:ms, :],
                                         wt[ci][0:cs, m0:m0 + ms],
                                         ones[0:cs, :],
                                         start=(ci == 0), stop=(ci == NCC - 1))
                    inv = persist.tile([128, 1], F32, tag=f"inv{mi}")
                    nc.vector.reciprocal(out=inv[0:ms, :], in_=ps_sum[0:ms, :])
                    invs.append(inv)
                return wt, invs

            wts, invss = [None] * BS, [None] * BS
            emit_emb(0, 1)
            wts[0], invss[0] = emit_weight_stage(0)
            prefetch_megs(0)
            emit_emb(1, BS - 1)
            for b in range(1, BS):
                wts[b], invss[b] = emit_weight_stage(b)

            # ---- phase 2: big matmuls, PE back-to-back -----------------
            for b in range(BS):
                wt, invs = wts[b], invss[b]
                megs = load_megs(b)
                for th in range(T // TH):
                    t0 = th * TH
                    for mi, (m0, ms) in enumerate(M_CHUNKS):
                        ot = outp.tile([ms, TH], F16, tag=f"ot{mi}")
                        ps_list = []
                        for tq in range(NT_Q):
                            ps_o = bps.tile([128, 512], F32, tag="ps_o")
                            ps_list.append(ps_o)
                            for ci, (c0, cs) in enumerate(CCF):
                                nc.tensor.matmul(
                                    ps_o[0:ms, :],
                                    wt[ci][0:cs, m0:m0 + ms],
                                    megs[ci][:, t0 + tq * 512:
                                             t0 + (tq + 1) * 512],
                                    start=(ci == 0),
                                    stop=(not K1_WAVE and ci == NF - 1))
                        if K1_WAVE:
                            # single-channel contribution: 4 concurrent
                            # row-tiled K=1 matmuls (one per t-chunk)
                            for tq in range(NT_Q):
                                nc.tensor.matmul(
                                    ps_list[tq][0:ms, :],
                                    wt[-1][32 * tq:32 * tq + 1, m0:m0 + ms],
                                    megs[-1][32 * tq:32 * tq + 1,
                                             th * 512:(th + 1) * 512],
                                    start=False, stop=True,
                                    tile_position=(32 * tq, 0))
                        for tq in range(NT_Q):
                            # scaled psum->sbuf copy; alternate DVE/ACT so
                            # neither engine becomes the bottleneck
                            if tq % 2 == 0:
                                nc.vector.tensor_scalar_mul(
                                    out=ot[:, tq * 512:(tq + 1) * 512],
                                    in0=ps_list[tq][0:ms, :],
                                    scalar1=invs[mi][0:ms, :])
                            else:
                                nc.scalar.activation(
                                    out=ot[:, tq * 512:(tq + 1) * 512],
                                    in_=ps_list[tq][0:ms, :], func=Copy,
                                    scale=invs[mi][0:ms, :])
                        if b == BS - 1 and th == T // TH - 1:
                            nc.gpsimd.dma_start(
                                out=out_d[b, m0:m0 + ms, t0:t0 + TH // 2],
                                in_=ot[:, 0:TH // 2])
                            nc.gpsimd.dma_start(
                                out=out_d[b, m0:m0 + ms,
                                          t0 + TH // 2:t0 + TH],
                                in_=ot[:, TH // 2:TH])
                        else:
                            nc.gpsimd.dma_start(
                                out=out_d[b, m0:m0 + ms, t0:t0 + TH], in_=ot)

    nc.compile()
    return nc


def _get_nc(key):
    if key not in _NC_CACHE:
        if key == "fast":
            _NC_CACHE[key] = _build_fast()
        else:
            _NC_CACHE[key] = _build_bass(key)
    return _NC_CACHE[key]


def _prep_host(meg, positions, subject_index, heads):
    """Build the 8 per-core input maps + pick the channel prefix length."""
    f32 = np.float32
    pos = np.asarray(positions, dtype=f32)
    a = ((pos[:, :, 0] + MARGIN) / WIDTH).astype(f32)           # [B, C]
    bcoord = ((pos[:, :, 1] + MARGIN) / WIDTH).astype(f32)      # [B, C]
    invalid = np.all(pos == INVALID, axis=-1)                   # [B, C]
    offs = np.where(invalid, f32(NEG_INF), f32(0.0)).astype(f32)

    # channels invalid in EVERY sample get weight exactly 0 (exp(-1e9)==0)
    # -> their meg data is never needed; use the valid prefix length
    valid_any = ~np.all(invalid, axis=0)                        # [C]
    c_used = int(np.max(np.nonzero(valid_any)[0])) + 1 if valid_any.any() else C

    h = np.asarray(heads, dtype=f32)[np.asarray(subject_index).astype(np.int64)]
    hT = np.ascontiguousarray(h.transpose(0, 2, 1))             # [B, 242, O]
    hta = np.concatenate(
        [hT[:, :NF2, :], np.ones((B, 1, CHOUT), dtype=f32)], axis=1)
    htb = np.ascontiguousarray(hT[:, NF2:, :])

    fr = np.arange(N_FREQS, dtype=f32)
    fi = np.repeat(fr, N_FREQS).reshape(NF2, 1)
    fj = np.tile(fr, N_FREQS).reshape(NF2, 1)
    import ml_dtypes as _mld
    ones = np.ones((128, 1), dtype=_mld.bfloat16)

    import ml_dtypes
    megf = np.asarray(meg, dtype=f32).astype(ml_dtypes.bfloat16)
    in_maps = []
    for c in range(N_CORES):
        s = slice(c * BS, (c + 1) * BS)
        in_maps.append(dict(
            meg=np.ascontiguousarray(megf[s]),
            pa=np.ascontiguousarray(a[s]),
            pb=np.ascontiguousarray(bcoord[s]),
            offs=np.ascontiguousarray(offs[s]),
            hta=np.ascontiguousarray(hta[s]),
            htb=np.ascontiguousarray(htb[s]),
            fi=fi, fj=fj, ones=ones,
        ))
    return in_maps, c_used


def kernel(meg, positions, subject_index, heads, _trace=False):
    from concourse.bass_utils import run_bass_kernel_spmd

    if _fast_path_ok(meg, positions, subject_index, heads):
        in_maps = _prep_host_fast(meg, positions, subject_index, heads)
        nc = _get_nc("fast")
    else:
        in_maps, c_used = _prep_host(meg, positions, subject_index, heads)
        nc = _get_nc(c_used)
    res = run_bass_kernel_spmd(nc, in_maps, core_ids=list(range(N_CORES)),
                               trace=_trace)
    out = np.concatenate([r["out"] for r in res.results], axis=0)
    if _trace:
        kernel.last_exec_time_ns = res.exec_time_ns
        kernel.last_results = res
    return out.astype(np.float32)
